# revision 40
# baseline (speedup 1.0000x reference)
"""Trainium2 Bass kernel for nn_AttnBlock_7954279432551.

Data-parallel over batch: B=16 across 8 NeuronCores (2 batch elems per core),
conv/fc weights replicated.  All matmuls run as float32r (FP22 mantissa,
full PE rate at free-dim >= 256).

Math per batch element (C=512, L=1024, O=1024, K=5, DW=512, P=196 pad 256):
  w = weight_norm(conv_v, conv_g)                          # host
  y[o,t]  = sum_{c,k} w[o,c,k] x[c, t+k-4] + conv_b        # conv, t in [0,1024)
  h       = y[0:512] * sigmoid(y[512:1024])                # GLU
  g[c,p]  = sum_a fc1_w[a,c] feat[a,p]                     # fused fc1 (feat = img_conv)
  s[l,p]  = sum_c h[c,l] g[c,p] + sum_a (we+fc1_b)[l,a] feat[a,p]
  attn    = softmax_p(s)
  ffwT[p,c]= sum_a feat[a,p] fc2_w[c,a]                    # fused fc2
  oT[c,l] = sum_p ffwT[p,c] attnT[p,l] + fc2_b[c]
  out     = oT + h + x
Returns (out, word_embed, img_conv, attn).
"""

import numpy as np
from contextlib import ExitStack

B, CIN, L = 16, 512, 1024
COUT, K = 1024, 5
DW, HW = 512, 196
PPAD = 256
NCORES = 8
BL = B // NCORES  # batch elems per core

_CACHE = {}


def _build():
    if "nc" in _CACHE:
        return _CACHE["nc"], _CACHE["tin"], _CACHE["tout"]
    import concourse.bass as bass
    import concourse.tile as tile
    from concourse import bacc, mybir

    nc = bacc.Bacc(
        "TRN2",
        target_bir_lowering=False,
        debug=False,
        enable_asserts=True,
        num_devices=NCORES,
    )
    f32 = mybir.dt.float32
    f32r = mybir.dt.float32r

    tin = {
        # x, host-padded with 4 leading zero columns
        "x_d": nc.dram_tensor("x_d", [BL, CIN, L + 4], f32r, kind="ExternalInput").ap(),
        # (word_embed + fc1_b) transposed to [DW, L]
        "weT_d": nc.dram_tensor("weT_d", [BL, DW, L], f32r, kind="ExternalInput").ap(),
        # img_conv as [DW, PPAD], zero-padded past HW
        "feat_d": nc.dram_tensor("feat_d", [BL, DW, PPAD], f32r, kind="ExternalInput").ap(),
        # conv weights, normalized, laid out [pair, cblk, half, cin%128, k, o%128]
        "wt_d": nc.dram_tensor("wt_d", [4, 4, 2, 128, K, 128], f32r, kind="ExternalInput").ap(),
        "fc1w_d": nc.dram_tensor("fc1w_d", [DW, DW], f32r, kind="ExternalInput").ap(),  # [a, c]
        "fc2wT_d": nc.dram_tensor("fc2wT_d", [DW, DW], f32r, kind="ExternalInput").ap(),  # [a, c2]
        # packed per-partition consts: [cba(4) | cbb(4) | fc2b(4)]
        "cc_d": nc.dram_tensor("cc_d", [128, 12], f32, kind="ExternalInput").ap(),
        "ident_d": nc.dram_tensor("ident_d", [128, 128], f32r, kind="ExternalInput").ap(),
    }
    tout = {
        "out_d": nc.dram_tensor("out_d", [BL, CIN, L], f32, kind="ExternalOutput").ap(),
        "attn_d": nc.dram_tensor("attn_d", [BL, L, HW], f32r, kind="ExternalOutput").ap(),
    }

    with tile.TileContext(nc) as tc:
        _emit(tc, tin, tout)
    nc.compile()

    _CACHE["nc"], _CACHE["tin"], _CACHE["tout"] = nc, tin, tout
    return nc, tin, tout


def _emit(tc, tin, tout):
    import concourse.bass as bass
    from concourse import mybir

    nc = tc.nc
    f32 = mybir.dt.float32
    f32r = mybir.dt.float32r
    AX = mybir.AxisListType.X
    AF = mybir.ActivationFunctionType
    OP = mybir.AluOpType
    PSUM = bass.MemorySpace.PSUM
    L4 = L + 4

    with ExitStack() as ctx:
        pw = ctx.enter_context(tc.tile_pool(name="pw", bufs=1))
        px = ctx.enter_context(tc.tile_pool(name="px", bufs=2))
        ph = ctx.enter_context(tc.tile_pool(name="ph", bufs=2))
        pwe = ctx.enter_context(tc.tile_pool(name="pwe", bufs=2))
        pfeat = ctx.enter_context(tc.tile_pool(name="pfeat", bufs=2))
        pg = ctx.enter_context(tc.tile_pool(name="pg", bufs=2))
        pffw = ctx.enter_context(tc.tile_pool(name="pffw", bufs=2))
        psig = ctx.enter_context(tc.tile_pool(name="psig", bufs=4))
        pst = ctx.enter_context(tc.tile_pool(name="pst", bufs=8))

        cc_s = pw.tile([128, 12], f32, tag="cc", name="cc_s")
        ident_s = pw.tile([128, 128], f32r, tag="id", name="ident_s")
        cba = lambda i: cc_s[:, i : i + 1]
        cbb = lambda i: cc_s[:, 4 + i : 5 + i]
        fc2b = lambda i: cc_s[:, 8 + i : 9 + i]
        ident = ident_s[:, :]
        fc1w_s = pw.tile([128, 4 * DW], f32r, tag="fc1w", name="fc1w_s")  # [a%, (ablk,c)]
        fc2w_s = pw.tile([128, 4 * DW], f32r, tag="fc2w", name="fc2w_s")  # [a%, (ablk,c2)]
        x_s = [px.tile([128, 4 * L4], f32r, tag="x", name=f"x{e}") for e in range(BL)]
        h_s = [ph.tile([128, 4 * L], f32r, tag="h", name=f"h{e}") for e in range(BL)]
        g_s = [pg.tile([128, 4, PPAD], f32r, tag="g", name=f"g{e}") for e in range(BL)]
        ffw_s = [pffw.tile([128, 2, DW], f32r, tag="ffw", name=f"ffw{e}") for e in range(BL)]
        feat_s = [pfeat.tile([128, 4, PPAD], f32r, tag="ft", name=f"ft{e}") for e in range(BL)]
        weT_s = [pwe.tile([128, 4 * L], f32r, tag="we", name=f"we{e}") for e in range(BL)]

        # conv phase: weight slabs live only here; pout reuses the space after
        with tc.tile_pool(name="pslab", bufs=8) as pslab, \
             tc.tile_pool(name="ppc", bufs=8, space=PSUM) as ppc:
            # DMA issue order = arrival order (transfers serialize on the DMA
            # bus), so interleave exactly in first-use order.
            slab0 = [None] * 8
            for cb in range(4):
                if cb == 0:
                    # split the very first transfer so the first matmul's
                    # operand window (cols 0..516) lands as early as possible
                    nc.sync.dma_start(
                        x_s[0][:, 0:516], tin["x_d"][0, 0:128, 0:516]
                    )
                    sl = pslab.tile([128, K, 128], f32r, tag="wsl", name="wsl000")
                    nc.sync.dma_start(sl[:, :, :], tin["wt_d"][0, 0, 0])
                    slab0[0] = sl
                    nc.sync.dma_start(
                        x_s[0][:, 516:L4], tin["x_d"][0, 0:128, 516:L4]
                    )
                    continue
                nc.sync.dma_start(
                    x_s[0][:, cb * L4 : (cb + 1) * L4],
                    tin["x_d"][0, cb * 128 : (cb + 1) * 128, :],
                )
                sl = pslab.tile([128, K, 128], f32r, tag="wsl", name=f"wsl0{cb}0")
                nc.sync.dma_start(sl[:, :, :], tin["wt_d"][0, cb, 0])
                slab0[cb * 2] = sl
            for cb in range(4):
                sl = pslab.tile([128, K, 128], f32r, tag="wsl", name=f"wsl0{cb}1")
                nc.sync.dma_start(sl[:, :, :], tin["wt_d"][0, cb, 1])
                slab0[cb * 2 + 1] = sl
            nc.sync.dma_start(cc_s[:, :], tin["cc_d"][:, :])
            nc.sync.dma_start(ident_s[:, :], tin["ident_d"][:, :])
            for cb in range(4):
                nc.sync.dma_start(
                    x_s[1][:, cb * L4 : (cb + 1) * L4],
                    tin["x_d"][1, cb * 128 : (cb + 1) * 128, :],
                )

            for i in range(4):  # GLU pair: o-tile i (a half) with o-tile i+4 (b half)
                if i == 0:
                    slabs = slab0
                else:
                    slabs = []
                    for cb in range(4):
                        for hh in range(2):
                            sl = pslab.tile([128, K, 128], f32r, tag="wsl", name=f"wsl{i}{cb}{hh}")
                            nc.sync.dma_start(sl[:, :, :], tin["wt_d"][i, cb, hh])
                            slabs.append(sl)
                for e in range(BL):
                    ya = {}
                    for hh in range(2):
                        for t0 in (0, 512):
                            y = ppc.tile([128, 512], f32, tag="y", name=f"y{i}{e}{t0}{hh}")
                            n = 0
                            for cb in range(4):
                                for k in range(K):
                                    nc.tensor.matmul(
                                        y[:, :],
                                        slabs[cb * 2 + hh][:, k, :],
                                        x_s[e][:, cb * L4 + t0 + k : cb * L4 + t0 + k + 512],
                                        start=(n == 0),
                                        stop=(n == 4 * K - 1),
                                    )
                                    n += 1
                            if hh == 0:
                                ya[t0] = y
                            else:
                                sig = psig.tile([128, 512], f32, tag="sig", name=f"sg{i}{e}{t0}")
                                nc.scalar.activation(sig[:, :], y[:, :], AF.Sigmoid, bias=cbb(i))
                                # h = (y_a + conv_b_a) * sigmoid(y_b + conv_b_b)
                                nc.vector.scalar_tensor_tensor(
                                    h_s[e][:, i * L + t0 : i * L + t0 + 512],
                                    ya[t0][:, :],
                                    cba(i),
                                    sig[:, :],
                                    op0=OP.add,
                                    op1=OP.mult,
                                )

            # attention-phase weights stream in behind the conv slabs
            for ab in range(4):
                nc.sync.dma_start(
                    feat_s[0][:, ab, :], tin["feat_d"][0, ab * 128 : (ab + 1) * 128, :]
                )
                nc.sync.dma_start(
                    fc1w_s[:, ab * DW : (ab + 1) * DW],
                    tin["fc1w_d"][ab * 128 : (ab + 1) * 128, :],
                )
                nc.sync.dma_start(
                    fc2w_s[:, ab * DW : (ab + 1) * DW],
                    tin["fc2wT_d"][ab * 128 : (ab + 1) * 128, :],
                )
            for ab in range(4):
                nc.sync.dma_start(
                    feat_s[1][:, ab, :], tin["feat_d"][1, ab * 128 : (ab + 1) * 128, :]
                )
                nc.sync.dma_start(
                    weT_s[0][:, ab * L : (ab + 1) * L],
                    tin["weT_d"][0, ab * 128 : (ab + 1) * 128, :],
                )
            for ab in range(4):
                nc.sync.dma_start(
                    weT_s[1][:, ab * L : (ab + 1) * L],
                    tin["weT_d"][1, ab * 128 : (ab + 1) * 128, :],
                )

            # g[c, p] = sum_a fc1_w[a, c] feat[a, p]  and
            # ffwT[p, c2] = sum_a feat[a, p] fc2_w[c2, a].  Elem 0's are at
            # the conv tail (needed first); elem 1's are woven into elem 0's
            # attention rounds where the PE has chain-latency gaps.
            for e in range(1):
                for cb in range(4):
                    gp = ppc.tile([128, PPAD], f32, tag="y", name=f"gp{e}{cb}")
                    for ab in range(4):
                        nc.tensor.matmul(
                            gp[:, :],
                            fc1w_s[:, ab * DW + cb * 128 : ab * DW + cb * 128 + 128],
                            feat_s[e][:, ab, :],
                            start=(ab == 0),
                            stop=(ab == 3),
                        )
                    nc.vector.tensor_copy(g_s[e][:, cb, :], gp[:, :])
                for hp, pw_ in ((0, 128), (1, 68)):
                    fp = ppc.tile([128, DW], f32, tag="y", name=f"fp{e}{hp}")
                    for ab in range(4):
                        nc.tensor.matmul(
                            fp[0:pw_, :],
                            feat_s[e][:, ab, hp * 128 : hp * 128 + pw_],
                            fc2w_s[:, ab * DW : (ab + 1) * DW],
                            start=(ab == 0),
                            stop=(ab == 3),
                        )
                    nc.vector.tensor_copy(ffw_s[e][0:pw_, hp, :], fp[0:pw_, :])

            # hx = h + x written over x_s (x fully consumed by the conv).
            # GpSimd is otherwise idle; keeps DVE clear for softmax + copies.
            for e in range(BL):
                for cb in range(4):
                    xs = x_s[e][:, cb * L4 + 4 : (cb + 1) * L4]
                    nc.gpsimd.tensor_add(xs, xs, h_s[e][:, cb * L : (cb + 1) * L])

        # ---- attention, per batch elem ----
        with (
            tc.tile_pool(name="pat", bufs=2) as pat,
            tc.tile_pool(name="patT", bufs=2) as patT,
            tc.tile_pool(name="pout", bufs=3) as pout,
            tc.tile_pool(name="pps", bufs=4, space=PSUM) as pps,
            tc.tile_pool(name="pptp", bufs=2, space=PSUM) as pptp,
            tc.tile_pool(name="ppol", bufs=2, space=PSUM) as ppol,
        ):
            aT_l = {}

            def emit_oT_half(e, c2, t0):
                # oT[c2, l] = sum_p ffwT[p, c2] attnT[p, l];
                # out = oT + fc2_b + hx  (hx = h + x, precomputed into x_s)
                op_ = ppol.tile([128, 512], f32, tag="o", name=f"op{e}{c2}{t0}")
                for hp, pw_ in ((0, 128), (1, 68)):
                    nc.tensor.matmul(
                        op_[:, :],
                        ffw_s[e][0:pw_, hp, c2 * 128 : c2 * 128 + 128],
                        aT_l[e][0:pw_, hp * L + t0 : hp * L + t0 + 512],
                        start=(hp == 0),
                        stop=(hp == 1),
                    )
                ot = pout.tile([128, 512], f32, tag="out", name=f"ot{e}{c2}{t0}")
                nc.vector.scalar_tensor_tensor(
                    ot[:, :],
                    op_[:, :],
                    fc2b(c2),
                    x_s[e][:, c2 * L4 + 4 + t0 : c2 * L4 + 4 + t0 + 512],
                    op0=OP.add,
                    op1=OP.add,
                )
                nc.sync.dma_start(
                    tout["out_d"][e, c2 * 128 : (c2 + 1) * 128, t0 : t0 + 512],
                    ot[:, :],
                )

            def emit_g_group(e, cb):
                gp = ppol.tile([128, PPAD], f32, tag="o", name=f"gp{e}{cb}")
                for ab in range(4):
                    nc.tensor.matmul(
                        gp[:, :],
                        fc1w_s[:, ab * DW + cb * 128 : ab * DW + cb * 128 + 128],
                        feat_s[e][:, ab, :],
                        start=(ab == 0),
                        stop=(ab == 3),
                    )
                nc.vector.tensor_copy(g_s[e][:, cb, :], gp[:, :])

            def emit_ffw_group(e, hp):
                pw_ = 128 if hp == 0 else 68
                fp = ppol.tile([128, DW], f32, tag="o", name=f"fp{e}{hp}")
                for ab in range(4):
                    nc.tensor.matmul(
                        fp[0:pw_, :],
                        feat_s[e][:, ab, hp * 128 : hp * 128 + pw_],
                        fc2w_s[:, ab * DW : (ab + 1) * DW],
                        start=(ab == 0),
                        stop=(ab == 3),
                    )
                nc.vector.tensor_copy(ffw_s[e][0:pw_, hp, :], fp[0:pw_, :])

            pend = []
            for e in range(BL):
                attn_s = pat.tile([128, 8 * PPAD], f32r, tag="at", name=f"at{e}")
                aT_s = patT.tile([128, 2 * L], f32r, tag="aT", name=f"aT{e}")
                aT_l[e] = aT_s

                def transposes(lb):
                    for hp, pw_ in ((0, 128), (1, 68)):
                        tp = pptp.tile([128, 128], f32r, tag="t", name=f"tp{e}{lb}{hp}")
                        nc.tensor.transpose(
                            tp[0:pw_, 0:128],
                            attn_s[:, lb * PPAD + hp * 128 : lb * PPAD + hp * 128 + pw_],
                            ident,
                        )
                        dst = aT_s[0:pw_, hp * L + lb * 128 : hp * L + lb * 128 + 128]
                        if hp == 0:
                            nc.vector.tensor_copy(dst, tp[0:pw_, 0:128])
                        else:
                            # DVE is the busiest attn-phase engine; ACT has slack
                            nc.scalar.copy(dst, tp[0:pw_, 0:128])

                # score + softmax; transposes delayed 3 lbs so the in-order PE
                # never waits on the softmax chain.  Chain-latency gaps carry
                # deferred oT halves (prev elem) and the next elem's g/ffw.
                for lb in range(8):
                    sp = pps.tile([128, PPAD], f32, tag="s", name=f"sp{e}{lb}")
                    for cb in range(4):
                        nc.tensor.matmul(
                            sp[:, :],
                            h_s[e][:, cb * L + lb * 128 : cb * L + lb * 128 + 128],
                            g_s[e][:, cb, :],
                            start=(cb == 0),
                            stop=False,
                        )
                    for ab in range(4):
                        nc.tensor.matmul(
                            sp[:, :],
                            weT_s[e][:, ab * L + lb * 128 : ab * L + lb * 128 + 128],
                            feat_s[e][:, ab, :],
                            start=False,
                            stop=(ab == 3),
                        )
                    nmax = pst.tile([128, 1], f32, tag="st", name=f"nm{e}{lb}")
                    nc.vector.reduce_max(nmax[:, :], sp[:, 0:HW], axis=AX, negate=True)
                    esum = pst.tile([128, 1], f32, tag="st", name=f"es{e}{lb}")
                    etmp = psig.tile([128, 512], f32, tag="sig", name=f"et{e}{lb}")
                    nc.scalar.activation(
                        etmp[:, 0:HW], sp[:, 0:HW], AF.Exp, bias=nmax[:, :], accum_out=esum[:, :]
                    )
                    rcp = pst.tile([128, 1], f32, tag="st", name=f"rc{e}{lb}")
                    nc.vector.reciprocal(rcp[:, :], esum[:, :])
                    nc.scalar.mul(attn_s[:, lb * PPAD : lb * PPAD + HW], etmp[:, 0:HW], rcp[:, :])
                    nc.sync.dma_start(
                        tout["attn_d"][e, lb * 128 : (lb + 1) * 128, :],
                        attn_s[:, lb * PPAD : lb * PPAD + HW],
                    )
                    if pend:
                        emit_oT_half(*pend.pop(0))
                    elif e + 1 < BL:
                        # next elem's fused-weight products fill this elem's
                        # early rounds
                        if lb < 4:
                            emit_g_group(e + 1, lb)
                        elif lb < 6:
                            emit_ffw_group(e + 1, lb - 4)
                    if lb > 2:
                        transposes(lb - 3)
                    if lb >= 6:
                        # rows 0..3 of aT are transposed by now: this elem's
                        # t0=0 halves can start
                        emit_oT_half(e, lb - 6, 0)
                for lbt in (5, 6, 7):
                    transposes(lbt)
                emit_oT_half(e, 2, 0)
                emit_oT_half(e, 3, 0)
                # remaining halves: run during the next elem's lb loop, or
                # directly if this is the last elem
                if e == BL - 1:
                    for c2 in range(4):
                        emit_oT_half(e, c2, 512)
                else:
                    for c2 in range(4):
                        pend.append((e, c2, 512))


def _prep(inputs):
    """Host-side input marshaling: weight norm + layouts. Returns in_maps."""
    x = np.asarray(inputs["x"], dtype=np.float32)
    we = np.asarray(inputs["word_embed"], dtype=np.float32)
    img = np.asarray(inputs["img_conv"], dtype=np.float32)
    conv_v = np.asarray(inputs["conv_v"], dtype=np.float64)
    conv_g = np.asarray(inputs["conv_g"], dtype=np.float64)
    conv_b = np.asarray(inputs["conv_b"], dtype=np.float32)
    fc1_w = np.asarray(inputs["fc1_w"], dtype=np.float32)
    fc1_b = np.asarray(inputs["fc1_b"], dtype=np.float32)
    fc2_w = np.asarray(inputs["fc2_w"], dtype=np.float32)
    fc2_b = np.asarray(inputs["fc2_b"], dtype=np.float32)

    vnorm = np.sqrt((conv_v * conv_v).sum(axis=(1, 2)))
    w = ((conv_g / vnorm)[:, None, None] * conv_v).astype(np.float32)  # [O, C, K]
    # -> [pair, cblk, half, c%128, k, o%128]
    w8 = w.reshape(2, 4, 128, 4, 128, K)  # [half, pair, o%128, cblk, c%128, k]
    wt = np.ascontiguousarray(w8.transpose(1, 3, 0, 4, 5, 2))

    weT = np.ascontiguousarray((we + fc1_b[None, None, :]).transpose(0, 2, 1))
    feat = img.reshape(B, DW, HW)
    xp = np.zeros((B, CIN, L + 4), dtype=np.float32)
    xp[:, :, 4:] = x
    featp = np.zeros((B, DW, PPAD), dtype=np.float32)
    featp[:, :, :HW] = feat

    cc = np.concatenate(
        [
            np.ascontiguousarray(conv_b[:DW].reshape(4, 128).T),
            np.ascontiguousarray(conv_b[DW:].reshape(4, 128).T),
            np.ascontiguousarray(fc2_b.reshape(4, 128).T),
        ],
        axis=1,
    ).astype(np.float32)
    cc = np.ascontiguousarray(cc)

    shared = {
        "wt_d": wt,
        "fc1w_d": np.ascontiguousarray(fc1_w),
        "fc2wT_d": np.ascontiguousarray(fc2_w.T),
        "cc_d": cc,
        "ident_d": np.eye(128, dtype=np.float32),
    }
    in_maps = []
    for c in range(NCORES):
        s = slice(c * BL, (c + 1) * BL)
        m = dict(shared)
        m["x_d"] = xp[s]
        m["weT_d"] = weT[s]
        m["feat_d"] = featp[s]
        in_maps.append(m)
    return in_maps


def _run(in_maps, trace=False):
    from concourse.bass_utils import run_bass_kernel_spmd

    nc, _, _ = _build()
    return run_bass_kernel_spmd(nc, in_maps, core_ids=list(range(NCORES)), trace=trace)


def kernel(**inputs):
    in_maps = _prep(inputs)
    res = _run(in_maps, trace=False)
    out = np.concatenate([res.results[c]["out_d"] for c in range(NCORES)], axis=0)
    attn = np.concatenate([res.results[c]["attn_d"] for c in range(NCORES)], axis=0)
    word_embed = np.asarray(inputs["word_embed"], dtype=np.float32)
    img_conv = np.asarray(inputs["img_conv"], dtype=np.float32)
    return (out, word_embed, img_conv, attn)


# revision 46
# speedup vs baseline: 1.0003x; 1.0003x over previous
"""Trainium2 Bass kernel for nn_AttnBlock_7954279432551.

Data-parallel over batch: B=16 across 8 NeuronCores (2 batch elems per core),
conv/fc weights replicated.  All matmuls run as float32r (FP22 mantissa,
full PE rate at free-dim >= 256).

Math per batch element (C=512, L=1024, O=1024, K=5, DW=512, P=196 pad 256):
  w = weight_norm(conv_v, conv_g)                          # host
  y[o,t]  = sum_{c,k} w[o,c,k] x[c, t+k-4] + conv_b        # conv, t in [0,1024)
  h       = y[0:512] * sigmoid(y[512:1024])                # GLU
  g[c,p]  = sum_a fc1_w[a,c] feat[a,p]                     # fused fc1 (feat = img_conv)
  s[l,p]  = sum_c h[c,l] g[c,p] + sum_a (we+fc1_b)[l,a] feat[a,p]
  attn    = softmax_p(s)
  ffwT[p,c]= sum_a feat[a,p] fc2_w[c,a]                    # fused fc2
  oT[c,l] = sum_p ffwT[p,c] attnT[p,l] + fc2_b[c]
  out     = oT + h + x
Returns (out, word_embed, img_conv, attn).
"""

import numpy as np
from contextlib import ExitStack

B, CIN, L = 16, 512, 1024
COUT, K = 1024, 5
DW, HW = 512, 196
PPAD = 256
NCORES = 8
BL = B // NCORES  # batch elems per core

_CACHE = {}


def _build():
    if "nc" in _CACHE:
        return _CACHE["nc"], _CACHE["tin"], _CACHE["tout"]
    import concourse.bass as bass
    import concourse.tile as tile
    from concourse import bacc, mybir

    nc = bacc.Bacc(
        "TRN2",
        target_bir_lowering=False,
        debug=False,
        enable_asserts=True,
        num_devices=NCORES,
    )
    f32 = mybir.dt.float32
    f32r = mybir.dt.float32r

    tin = {
        # x, host-padded with 4 leading zero columns
        "x_d": nc.dram_tensor("x_d", [BL, CIN, L + 4], f32r, kind="ExternalInput").ap(),
        # (word_embed + fc1_b) transposed to [DW, L]
        "weT_d": nc.dram_tensor("weT_d", [BL, DW, L], f32r, kind="ExternalInput").ap(),
        # img_conv as [DW, PPAD], zero-padded past HW
        "feat_d": nc.dram_tensor("feat_d", [BL, DW, PPAD], f32r, kind="ExternalInput").ap(),
        # conv weights, normalized, laid out [pair, cblk, half, cin%128, k, o%128]
        "wt_d": nc.dram_tensor("wt_d", [4, 4, 2, 128, K, 128], f32r, kind="ExternalInput").ap(),
        "fc1w_d": nc.dram_tensor("fc1w_d", [DW, DW], f32r, kind="ExternalInput").ap(),  # [a, c]
        "fc2wT_d": nc.dram_tensor("fc2wT_d", [DW, DW], f32r, kind="ExternalInput").ap(),  # [a, c2]
        # packed per-partition consts: [cba(4) | cbb(4) | fc2b(4)]
        "cc_d": nc.dram_tensor("cc_d", [128, 12], f32, kind="ExternalInput").ap(),
        "ident_d": nc.dram_tensor("ident_d", [128, 128], f32r, kind="ExternalInput").ap(),
    }
    tout = {
        "out_d": nc.dram_tensor("out_d", [BL, CIN, L], f32, kind="ExternalOutput").ap(),
        "attn_d": nc.dram_tensor("attn_d", [BL, L, HW], f32r, kind="ExternalOutput").ap(),
    }

    with tile.TileContext(nc) as tc:
        _emit(tc, tin, tout)
    nc.compile()

    _CACHE["nc"], _CACHE["tin"], _CACHE["tout"] = nc, tin, tout
    return nc, tin, tout


def _emit(tc, tin, tout):
    import concourse.bass as bass
    from concourse import mybir

    nc = tc.nc
    f32 = mybir.dt.float32
    f32r = mybir.dt.float32r
    AX = mybir.AxisListType.X
    AF = mybir.ActivationFunctionType
    OP = mybir.AluOpType
    PSUM = bass.MemorySpace.PSUM
    L4 = L + 4

    with ExitStack() as ctx:
        pw = ctx.enter_context(tc.tile_pool(name="pw", bufs=1))
        px = ctx.enter_context(tc.tile_pool(name="px", bufs=2))
        ph = ctx.enter_context(tc.tile_pool(name="ph", bufs=2))
        pwe = ctx.enter_context(tc.tile_pool(name="pwe", bufs=2))
        pfeat = ctx.enter_context(tc.tile_pool(name="pfeat", bufs=2))
        pg = ctx.enter_context(tc.tile_pool(name="pg", bufs=2))
        pffw = ctx.enter_context(tc.tile_pool(name="pffw", bufs=2))
        psig = ctx.enter_context(tc.tile_pool(name="psig", bufs=4))
        pst = ctx.enter_context(tc.tile_pool(name="pst", bufs=8))

        cc_s = pw.tile([128, 12], f32, tag="cc", name="cc_s")
        ident_s = pw.tile([128, 128], f32r, tag="id", name="ident_s")
        cba = lambda i: cc_s[:, i : i + 1]
        cbb = lambda i: cc_s[:, 4 + i : 5 + i]
        fc2b = lambda i: cc_s[:, 8 + i : 9 + i]
        ident = ident_s[:, :]
        fc1w_s = pw.tile([128, 4 * DW], f32r, tag="fc1w", name="fc1w_s")  # [a%, (ablk,c)]
        fc2w_s = pw.tile([128, 4 * DW], f32r, tag="fc2w", name="fc2w_s")  # [a%, (ablk,c2)]
        x_s = [px.tile([128, 4 * L4], f32r, tag="x", name=f"x{e}") for e in range(BL)]
        h_s = [ph.tile([128, 4 * L], f32r, tag="h", name=f"h{e}") for e in range(BL)]
        g_s = [pg.tile([128, 4, PPAD], f32r, tag="g", name=f"g{e}") for e in range(BL)]
        ffw_s = [pffw.tile([128, 2, DW], f32r, tag="ffw", name=f"ffw{e}") for e in range(BL)]
        feat_s = [pfeat.tile([128, 4, PPAD], f32r, tag="ft", name=f"ft{e}") for e in range(BL)]
        weT_s = [pwe.tile([128, 4 * L], f32r, tag="we", name=f"we{e}") for e in range(BL)]

        # conv phase: weight slabs live only here; pout reuses the space after
        with tc.tile_pool(name="pslab", bufs=8) as pslab, \
             tc.tile_pool(name="ppc", bufs=8, space=PSUM) as ppc:
            # DMA issue order = arrival order (transfers serialize on the DMA
            # bus), so interleave exactly in first-use order.
            slab0 = [None] * 8
            for cb in range(4):
                if cb == 0:
                    # split the very first transfer so the first matmul's
                    # operand window (cols 0..516) lands as early as possible
                    nc.sync.dma_start(
                        x_s[0][:, 0:516], tin["x_d"][0, 0:128, 0:516]
                    )
                    sl = pslab.tile([128, K, 128], f32r, tag="wsl", name="wsl000")
                    nc.sync.dma_start(sl[:, :, :], tin["wt_d"][0, 0, 0])
                    slab0[0] = sl
                    nc.sync.dma_start(
                        x_s[0][:, 516:L4], tin["x_d"][0, 0:128, 516:L4]
                    )
                    continue
                nc.sync.dma_start(
                    x_s[0][:, cb * L4 : (cb + 1) * L4],
                    tin["x_d"][0, cb * 128 : (cb + 1) * 128, :],
                )
                sl = pslab.tile([128, K, 128], f32r, tag="wsl", name=f"wsl0{cb}0")
                nc.sync.dma_start(sl[:, :, :], tin["wt_d"][0, cb, 0])
                slab0[cb * 2] = sl
            for cb in range(4):
                sl = pslab.tile([128, K, 128], f32r, tag="wsl", name=f"wsl0{cb}1")
                nc.sync.dma_start(sl[:, :, :], tin["wt_d"][0, cb, 1])
                slab0[cb * 2 + 1] = sl
            nc.sync.dma_start(cc_s[:, :], tin["cc_d"][:, :])
            nc.sync.dma_start(ident_s[:, :], tin["ident_d"][:, :])
            for cb in range(4):
                nc.sync.dma_start(
                    x_s[1][:, cb * L4 : (cb + 1) * L4],
                    tin["x_d"][1, cb * 128 : (cb + 1) * 128, :],
                )

            for i in range(4):  # GLU pair: o-tile i (a half) with o-tile i+4 (b half)
                if i == 0:
                    slabs = slab0
                else:
                    slabs = []
                    for cb in range(4):
                        for hh in range(2):
                            sl = pslab.tile([128, K, 128], f32r, tag="wsl", name=f"wsl{i}{cb}{hh}")
                            nc.sync.dma_start(sl[:, :, :], tin["wt_d"][i, cb, hh])
                            slabs.append(sl)
                for e in range(BL):
                    ya = {}
                    for hh in range(2):
                        for t0 in (0, 512):
                            y = ppc.tile([128, 512], f32, tag="y", name=f"y{i}{e}{t0}{hh}")
                            n = 0
                            for cb in range(4):
                                for k in range(K):
                                    nc.tensor.matmul(
                                        y[:, :],
                                        slabs[cb * 2 + hh][:, k, :],
                                        x_s[e][:, cb * L4 + t0 + k : cb * L4 + t0 + k + 512],
                                        start=(n == 0),
                                        stop=(n == 4 * K - 1),
                                    )
                                    n += 1
                            if hh == 0:
                                ya[t0] = y
                            else:
                                sig = psig.tile([128, 512], f32, tag="sig", name=f"sg{i}{e}{t0}")
                                nc.scalar.activation(sig[:, :], y[:, :], AF.Sigmoid, bias=cbb(i))
                                # h = (y_a + conv_b_a) * sigmoid(y_b + conv_b_b)
                                nc.vector.scalar_tensor_tensor(
                                    h_s[e][:, i * L + t0 : i * L + t0 + 512],
                                    ya[t0][:, :],
                                    cba(i),
                                    sig[:, :],
                                    op0=OP.add,
                                    op1=OP.mult,
                                )

            # attention-phase weights stream in behind the conv slabs
            for ab in range(4):
                nc.sync.dma_start(
                    feat_s[0][:, ab, :], tin["feat_d"][0, ab * 128 : (ab + 1) * 128, :]
                )
                nc.sync.dma_start(
                    fc1w_s[:, ab * DW : (ab + 1) * DW],
                    tin["fc1w_d"][ab * 128 : (ab + 1) * 128, :],
                )
                nc.sync.dma_start(
                    fc2w_s[:, ab * DW : (ab + 1) * DW],
                    tin["fc2wT_d"][ab * 128 : (ab + 1) * 128, :],
                )
            for ab in range(4):
                nc.sync.dma_start(
                    feat_s[1][:, ab, :], tin["feat_d"][1, ab * 128 : (ab + 1) * 128, :]
                )
                nc.sync.dma_start(
                    weT_s[0][:, ab * L : (ab + 1) * L],
                    tin["weT_d"][0, ab * 128 : (ab + 1) * 128, :],
                )
            for ab in range(4):
                nc.sync.dma_start(
                    weT_s[1][:, ab * L : (ab + 1) * L],
                    tin["weT_d"][1, ab * 128 : (ab + 1) * 128, :],
                )

            # g[c, p] = sum_a fc1_w[a, c] feat[a, p]  and
            # ffwT[p, c2] = sum_a feat[a, p] fc2_w[c2, a].  Elem 0's are at
            # the conv tail (needed first); elem 1's are woven into elem 0's
            # attention rounds where the PE has chain-latency gaps.
            for e in range(1):
                for cb in range(4):
                    gp = ppc.tile([128, PPAD], f32, tag="y", name=f"gp{e}{cb}")
                    for ab in range(4):
                        nc.tensor.matmul(
                            gp[:, :],
                            fc1w_s[:, ab * DW + cb * 128 : ab * DW + cb * 128 + 128],
                            feat_s[e][:, ab, :],
                            start=(ab == 0),
                            stop=(ab == 3),
                        )
                    # ACT is idle at the conv tail (sigmoids done); DVE still
                    # drains GLU stts — and these copies gate the attn psum
                    # pool boundary
                    nc.scalar.copy(g_s[e][:, cb, :], gp[:, :])
                for hp, pw_ in ((0, 128), (1, 68)):
                    fp = ppc.tile([128, DW], f32, tag="y", name=f"fp{e}{hp}")
                    for ab in range(4):
                        nc.tensor.matmul(
                            fp[0:pw_, :],
                            feat_s[e][:, ab, hp * 128 : hp * 128 + pw_],
                            fc2w_s[:, ab * DW : (ab + 1) * DW],
                            start=(ab == 0),
                            stop=(ab == 3),
                        )
                    nc.scalar.copy(ffw_s[e][0:pw_, hp, :], fp[0:pw_, :])

            # hx = h + x written over x_s (x fully consumed by the conv).
            # GpSimd is otherwise idle; keeps DVE clear for softmax + copies.
            for e in range(BL):
                for cb in range(4):
                    xs = x_s[e][:, cb * L4 + 4 : (cb + 1) * L4]
                    nc.gpsimd.tensor_add(xs, xs, h_s[e][:, cb * L : (cb + 1) * L])

        # ---- attention, per batch elem ----
        with (
            tc.tile_pool(name="pat", bufs=2) as pat,
            tc.tile_pool(name="patT", bufs=2) as patT,
            tc.tile_pool(name="pout", bufs=3) as pout,
            tc.tile_pool(name="pps", bufs=4, space=PSUM) as pps,
            tc.tile_pool(name="pptp", bufs=2, space=PSUM) as pptp,
            tc.tile_pool(name="ppol", bufs=2, space=PSUM) as ppol,
        ):
            aT_l = {}

            def emit_oT_half(e, c2, t0):
                # oT[c2, l] = sum_p ffwT[p, c2] attnT[p, l];
                # out = oT + fc2_b + hx  (hx = h + x, precomputed into x_s)
                op_ = ppol.tile([128, 512], f32, tag="o", name=f"op{e}{c2}{t0}")
                for hp, pw_ in ((0, 128), (1, 68)):
                    nc.tensor.matmul(
                        op_[:, :],
                        ffw_s[e][0:pw_, hp, c2 * 128 : c2 * 128 + 128],
                        aT_l[e][0:pw_, hp * L + t0 : hp * L + t0 + 512],
                        start=(hp == 0),
                        stop=(hp == 1),
                    )
                ot = pout.tile([128, 512], f32, tag="out", name=f"ot{e}{c2}{t0}")
                nc.vector.scalar_tensor_tensor(
                    ot[:, :],
                    op_[:, :],
                    fc2b(c2),
                    x_s[e][:, c2 * L4 + 4 + t0 : c2 * L4 + 4 + t0 + 512],
                    op0=OP.add,
                    op1=OP.add,
                )
                nc.sync.dma_start(
                    tout["out_d"][e, c2 * 128 : (c2 + 1) * 128, t0 : t0 + 512],
                    ot[:, :],
                )

            def emit_g_group(e, cb):
                gp = ppol.tile([128, PPAD], f32, tag="o", name=f"gp{e}{cb}")
                for ab in range(4):
                    nc.tensor.matmul(
                        gp[:, :],
                        fc1w_s[:, ab * DW + cb * 128 : ab * DW + cb * 128 + 128],
                        feat_s[e][:, ab, :],
                        start=(ab == 0),
                        stop=(ab == 3),
                    )
                nc.vector.tensor_copy(g_s[e][:, cb, :], gp[:, :])

            def emit_ffw_group(e, hp):
                pw_ = 128 if hp == 0 else 68
                fp = ppol.tile([128, DW], f32, tag="o", name=f"fp{e}{hp}")
                for ab in range(4):
                    nc.tensor.matmul(
                        fp[0:pw_, :],
                        feat_s[e][:, ab, hp * 128 : hp * 128 + pw_],
                        fc2w_s[:, ab * DW : (ab + 1) * DW],
                        start=(ab == 0),
                        stop=(ab == 3),
                    )
                nc.vector.tensor_copy(ffw_s[e][0:pw_, hp, :], fp[0:pw_, :])

            pend = []
            for e in range(BL):
                attn_s = pat.tile([128, 8 * PPAD], f32r, tag="at", name=f"at{e}")
                aT_s = patT.tile([128, 2 * L], f32r, tag="aT", name=f"aT{e}")
                aT_l[e] = aT_s

                def transposes(lb):
                    for hp, pw_ in ((0, 128), (1, 68)):
                        tp = pptp.tile([128, 128], f32r, tag="t", name=f"tp{e}{lb}{hp}")
                        nc.tensor.transpose(
                            tp[0:pw_, 0:128],
                            attn_s[:, lb * PPAD + hp * 128 : lb * PPAD + hp * 128 + pw_],
                            ident,
                        )
                        dst = aT_s[0:pw_, hp * L + lb * 128 : hp * L + lb * 128 + 128]
                        if hp == 0:
                            nc.vector.tensor_copy(dst, tp[0:pw_, 0:128])
                        else:
                            # DVE is the busiest attn-phase engine; ACT has slack
                            nc.scalar.copy(dst, tp[0:pw_, 0:128])

                # score + softmax; transposes delayed 3 lbs so the in-order PE
                # never waits on the softmax chain.  Chain-latency gaps carry
                # deferred oT halves (prev elem) and the next elem's g/ffw.
                for lb in range(8):
                    sp = pps.tile([128, PPAD], f32, tag="s", name=f"sp{e}{lb}")
                    for cb in range(4):
                        nc.tensor.matmul(
                            sp[:, :],
                            h_s[e][:, cb * L + lb * 128 : cb * L + lb * 128 + 128],
                            g_s[e][:, cb, :],
                            start=(cb == 0),
                            stop=False,
                        )
                    for ab in range(4):
                        nc.tensor.matmul(
                            sp[:, :],
                            weT_s[e][:, ab * L + lb * 128 : ab * L + lb * 128 + 128],
                            feat_s[e][:, ab, :],
                            start=False,
                            stop=(ab == 3),
                        )
                    nmax = pst.tile([128, 1], f32, tag="st", name=f"nm{e}{lb}")
                    nc.vector.reduce_max(nmax[:, :], sp[:, 0:HW], axis=AX, negate=True)
                    esum = pst.tile([128, 1], f32, tag="st", name=f"es{e}{lb}")
                    etmp = psig.tile([128, 512], f32, tag="sig", name=f"et{e}{lb}")
                    nc.scalar.activation(
                        etmp[:, 0:HW], sp[:, 0:HW], AF.Exp, bias=nmax[:, :], accum_out=esum[:, :]
                    )
                    rcp = pst.tile([128, 1], f32, tag="st", name=f"rc{e}{lb}")
                    nc.vector.reciprocal(rcp[:, :], esum[:, :])
                    nc.scalar.mul(attn_s[:, lb * PPAD : lb * PPAD + HW], etmp[:, 0:HW], rcp[:, :])
                    nc.sync.dma_start(
                        tout["attn_d"][e, lb * 128 : (lb + 1) * 128, :],
                        attn_s[:, lb * PPAD : lb * PPAD + HW],
                    )
                    if pend:
                        emit_oT_half(*pend.pop(0))
                    elif e + 1 < BL:
                        # next elem's fused-weight products fill this elem's
                        # early rounds
                        if lb < 4:
                            emit_g_group(e + 1, lb)
                        elif lb < 6:
                            emit_ffw_group(e + 1, lb - 4)
                    if lb > 2:
                        transposes(lb - 3)
                    if lb >= 6:
                        # rows 0..3 of aT are transposed by now: this elem's
                        # t0=0 halves can start
                        emit_oT_half(e, lb - 6, 0)
                for lbt in (5, 6, 7):
                    transposes(lbt)
                emit_oT_half(e, 2, 0)
                emit_oT_half(e, 3, 0)
                # remaining halves: run during the next elem's lb loop, or
                # directly if this is the last elem
                if e == BL - 1:
                    for c2 in range(4):
                        emit_oT_half(e, c2, 512)
                else:
                    for c2 in range(4):
                        pend.append((e, c2, 512))


def _prep(inputs):
    """Host-side input marshaling: weight norm + layouts. Returns in_maps."""
    x = np.asarray(inputs["x"], dtype=np.float32)
    we = np.asarray(inputs["word_embed"], dtype=np.float32)
    img = np.asarray(inputs["img_conv"], dtype=np.float32)
    conv_v = np.asarray(inputs["conv_v"], dtype=np.float64)
    conv_g = np.asarray(inputs["conv_g"], dtype=np.float64)
    conv_b = np.asarray(inputs["conv_b"], dtype=np.float32)
    fc1_w = np.asarray(inputs["fc1_w"], dtype=np.float32)
    fc1_b = np.asarray(inputs["fc1_b"], dtype=np.float32)
    fc2_w = np.asarray(inputs["fc2_w"], dtype=np.float32)
    fc2_b = np.asarray(inputs["fc2_b"], dtype=np.float32)

    vnorm = np.sqrt((conv_v * conv_v).sum(axis=(1, 2)))
    w = ((conv_g / vnorm)[:, None, None] * conv_v).astype(np.float32)  # [O, C, K]
    # -> [pair, cblk, half, c%128, k, o%128]
    w8 = w.reshape(2, 4, 128, 4, 128, K)  # [half, pair, o%128, cblk, c%128, k]
    wt = np.ascontiguousarray(w8.transpose(1, 3, 0, 4, 5, 2))

    weT = np.ascontiguousarray((we + fc1_b[None, None, :]).transpose(0, 2, 1))
    feat = img.reshape(B, DW, HW)
    xp = np.zeros((B, CIN, L + 4), dtype=np.float32)
    xp[:, :, 4:] = x
    featp = np.zeros((B, DW, PPAD), dtype=np.float32)
    featp[:, :, :HW] = feat

    cc = np.concatenate(
        [
            np.ascontiguousarray(conv_b[:DW].reshape(4, 128).T),
            np.ascontiguousarray(conv_b[DW:].reshape(4, 128).T),
            np.ascontiguousarray(fc2_b.reshape(4, 128).T),
        ],
        axis=1,
    ).astype(np.float32)
    cc = np.ascontiguousarray(cc)

    shared = {
        "wt_d": wt,
        "fc1w_d": np.ascontiguousarray(fc1_w),
        "fc2wT_d": np.ascontiguousarray(fc2_w.T),
        "cc_d": cc,
        "ident_d": np.eye(128, dtype=np.float32),
    }
    in_maps = []
    for c in range(NCORES):
        s = slice(c * BL, (c + 1) * BL)
        m = dict(shared)
        m["x_d"] = xp[s]
        m["weT_d"] = weT[s]
        m["feat_d"] = featp[s]
        in_maps.append(m)
    return in_maps


def _run(in_maps, trace=False):
    from concourse.bass_utils import run_bass_kernel_spmd

    nc, _, _ = _build()
    return run_bass_kernel_spmd(nc, in_maps, core_ids=list(range(NCORES)), trace=trace)


def kernel(**inputs):
    in_maps = _prep(inputs)
    res = _run(in_maps, trace=False)
    out = np.concatenate([res.results[c]["out_d"] for c in range(NCORES)], axis=0)
    attn = np.concatenate([res.results[c]["attn_d"] for c in range(NCORES)], axis=0)
    word_embed = np.asarray(inputs["word_embed"], dtype=np.float32)
    img_conv = np.asarray(inputs["img_conv"], dtype=np.float32)
    return (out, word_embed, img_conv, attn)
